# revision 5
# baseline (speedup 1.0000x reference)
"""Multi-head self-attention (B=8, S=1024, E=1024, H=16) on 8 TRN2 cores.

Sharding: tensor-parallel on heads — core c owns heads (2c, 2c+1) for ALL
batches. Every core runs the identical program (SPMD); only the W/bias column
slices differ per core, so the load is balanced by construction.

Per-batch sequence truncation: batch b is processed only up to
L_b = ceil(l_b/128)*128 rows (compile-time constants baked per call, cached on
the value of l). Rows q >= l_b are discarded on the host; causal masking makes
the padded key columns inside the last tile harmless for valid rows.

x lives in HBM in a blocked layout [128 part][blk][e_t][256] so that any
span of 256-blocks is one DMA with 128 large contiguous descriptors (the
per-row descriptor storm of the naive [E, stot] layout was both dispatch-
and queue-bound).

Engine plan per core:
  PE : QKV projection (contraction over E in 8 tiles of 128), scores QK^T
       (contraction 64 -> two PE row-halves in parallel), out = E@V with the
       softmax denominator as a ones column of V.
  ACT: exp only (one table set, loaded once at start), two heads per
       ACTIVATE instruction to amortize the fixed overhead.
  DVE: PSUM->SBUF copies (QK bias-add, paired V tiles with the V-bias fused
       into the copy, out rows), causal diag mask per k-tile.
  Out rows carry the denominator (65 cols per head); division on the host.
Emission software-pipelines proj(b+1) and the first score tile of (b+1) into
attention(b) to cover ACT latency and batch transitions.
"""

import sys

sys.path.insert(0, "/opt/trn_rl_repo")

from collections import deque

import numpy as np
import ml_dtypes

import concourse.bass as bass
import concourse.bacc as bacc
import concourse.mybir as mybir
import concourse.tile as tile
from concourse.bass import ds, ts
from concourse.bass_utils import run_bass_kernel_spmd

P = 128
BLK = 256
B, S, E, H = 8, 1024, 1024, 16
DH = E // H  # 64
F32 = mybir.dt.float32
BF16 = mybir.dt.bfloat16
ETDT = BF16
NT = E // P  # 8 contraction tiles

_cached = {}


def _plan(l):
    lex = [min(int(v), S) for v in l]
    lpad = [min((int(v) + P - 1) // P * P, S) for v in l]
    l2 = [(Lp + BLK - 1) // BLK * BLK for Lp in lpad]
    order = sorted(range(B), key=lambda b: -lpad[b])
    offs = {}   # in positions, within the padded (l2) layout
    off = 0
    for b in order:
        offs[b] = off
        off += l2[b]
    return tuple(lpad), tuple(l2), tuple(order), offs, off, tuple(lex)


def _build_program(lpad, l2, order, offs, stot2, lex):
    nc = bacc.Bacc(None, target_bir_lowering=False)

    # x: [128][blk][e_t][256] blocked layout (see module docstring)
    xt2 = nc.dram_tensor(
        "xt2", [P, (stot2 // BLK) * NT * BLK], BF16, kind="ExternalInput")[:]
    # constsA: wqk Q columns [P, NT, 128]
    constsA = nc.dram_tensor("constsA", [P, NT * P], BF16,
                             kind="ExternalInput")[:]
    # constsB: wqk K columns [P, NT*128] | wv [P, NT*130] | cm2 [P, 2*128]
    CB_W = NT * P + NT * 130 + 2 * P
    constsB = nc.dram_tensor("constsB", [P, CB_W], BF16,
                             kind="ExternalInput")[:]
    # bconsts: f32: cols 0:2 = qk bias (q, k); 2:262 = V bias row
    # replicated across partitions (with 1.0 denominator columns)
    bconsts = nc.dram_tensor("bconsts", [P, 262], F32, kind="ExternalInput")[:]
    # out: per batch tile t: [P, 130] (two heads x (64 + denom))
    o = nc.dram_tensor("o", [P, sum(Lp // P for Lp in lpad) * 130], F32,
                       kind="ExternalOutput")[:]
    # output tile offsets (in 130-col units) per batch, in `order`
    otile = {}
    ot = 0
    for b in order:
        otile[b] = ot
        ot += lpad[b] // P

    with tile.TileContext(nc) as tc:
        from contextlib import ExitStack

        with ExitStack() as ctx:
            sb = ctx.enter_context(tc.tile_pool(name="sb", bufs=1))
            wqkq_sb = sb.tile([P, NT, P], BF16)
            constsB_sb = sb.tile([P, CB_W], BF16)
            wqkk_sb = constsB_sb[:, 0:NT * P].rearrange(
                "p (t i) -> p t i", t=NT)
            wv_sb = constsB_sb[:, ds(NT * P, NT * 130)].rearrange(
                "p (t i) -> p t i", t=NT)
            cm2_sb = constsB_sb[:, ds(NT * P + NT * 130, 2 * P)].rearrange(
                "p (h q) -> p h q", h=2)
            bconsts_sb = sb.tile([P, 262], F32)
            bqk_sb = bconsts_sb[:, 0:2]
            bvr_sb = bconsts_sb[:, 2:262]
            warm_sb = sb.tile([P, 8], F32)
            warm2_sb = sb.tile([P, 8], BF16)

            xp = ctx.enter_context(tc.tile_pool(name="xp", bufs=1))
            qkp = ctx.enter_context(tc.tile_pool(name="qkp", bufs=2))
            vpp = ctx.enter_context(tc.tile_pool(name="vpp", bufs=2))
            eTp = ctx.enter_context(tc.tile_pool(name="eTp", bufs=2))
            outp = ctx.enter_context(tc.tile_pool(name="outp", bufs=4))
            pjps = ctx.enter_context(
                tc.tile_pool(name="pjps", bufs=2, space="PSUM"))
            sps_p = ctx.enter_context(
                tc.tile_pool(name="sps", bufs=2, space="PSUM"))
            ops_p = ctx.enter_context(
                tc.tile_pool(name="ops", bufs=2, space="PSUM"))

            # Warm the ACT exp table ASAP (overlaps const DMAs + first x DMA).
            nc.vector.memset(warm_sb, 0.0)
            nc.scalar.activation(
                out=warm2_sb, in_=warm_sb,
                func=mybir.ActivationFunctionType.Exp, scale=1.0)

            xbuf = {}

            def emit_xdma(b, eng, spans):
                """spans: list of (blk0, nblk) in 256-block units."""
                nb2 = l2[b] // BLK
                if b not in xbuf:
                    xbuf[b] = xp.tile([P, nb2, NT, BLK], BF16,
                                      name=f"xb{b}", tag=f"xb{b}")
                xb = xbuf[b]
                xsrc = xt2.rearrange("p (k t s) -> p k t s", t=NT, s=BLK)
                base = offs[b] // BLK
                for (a, n) in spans:
                    eng.dma_start(
                        out=xb[:, ds(a, n), :, :],
                        in_=xsrc[:, ds(base + a, n), :, :])

            # ---- DMA schedule ----
            # sync: wqkq -> x(b0) first block -> constsB -> x(b0) rest
            #       -> bconsts -> x(b1); gpsimd: x(b2..b7).
            b0 = order[0]
            nb2_0 = l2[b0] // BLK
            nc.sync.dma_start(
                out=wqkq_sb, in_=constsA.rearrange("p (t i) -> p t i", t=NT))
            emit_xdma(b0, nc.sync, [(0, 1)])
            nc.sync.dma_start(out=constsB_sb, in_=constsB)
            if nb2_0 > 1:
                emit_xdma(b0, nc.sync, [(1, nb2_0 - 1)])
            nc.sync.dma_start(out=bconsts_sb, in_=bconsts)
            if len(order) > 1:
                emit_xdma(order[1], nc.sync, [(0, l2[order[1]] // BLK)])
            for b in order[2:]:
                emit_xdma(b, nc.gpsimd, [(0, l2[b] // BLK)])

            state = {}

            def qk_chunks(b):
                """(j, c0, cn) list; block-aligned starts, 512 preferred.
                The very first batch starts with a 256 chunk so compute can
                begin as soon as its first x block lands."""
                lq = lex[b]
                out = []
                for j, end in ((0, lq), (1, lpad[b])):
                    c0 = 0
                    while c0 < end:
                        cap = BLK if (b == order[0] and c0 == 0) else 512
                        cn = min(cap, end - c0)
                        out.append((j, c0, cn))
                        c0 += cn
                return out

            def proj_thunks(b):
                """Closures emitting proj(b); each ~0.5-2us of PE work."""
                L = lpad[b]
                T = L // P
                qkT = qkp.tile([P, 2, L], BF16, name="qkT")
                vp = vpp.tile([P, T, 2, DH + 1], BF16, name="vp")
                toff = [t * L - t * (t - 1) * P // 2 for t in range(T + 1)]
                eT = eTp.tile([P, 2, toff[T]], ETDT, name="eT")
                out_sb = outp.tile([P, T, 130], F32, name="out_sb")
                state[b] = (qkT, vp, eT, out_sb, toff)

                def qk_chunk(j, c0, cn):
                    def th():
                        xb = xbuf[b]
                        ps = pjps.tile([P, 512], F32, name="pps")
                        # moving operand: whole blocks, then ragged tail
                        nwhole = (cn // BLK) * BLK
                        for e_t in range(NT):
                            if nwhole:
                                nc.tensor.matmul(
                                    ps[:, 0:nwhole],
                                    lhsT=(wqkq_sb if j == 0 else wqkk_sb)[
                                        :, e_t, :],
                                    rhs=xb[:, ds(c0 // BLK, nwhole // BLK),
                                           e_t, :],
                                    start=(e_t == 0),
                                    stop=(e_t == NT - 1 and cn == nwhole))
                        for e_t in range(NT):
                            if cn > nwhole:
                                nc.tensor.matmul(
                                    ps[:, ds(nwhole, cn - nwhole)],
                                    lhsT=(wqkq_sb if j == 0 else wqkk_sb)[
                                        :, e_t, :],
                                    rhs=xb[:, (c0 + nwhole) // BLK, e_t,
                                           ds(0, cn - nwhole)],
                                    start=(e_t == 0 and nwhole == 0),
                                    stop=(e_t == NT - 1),
                                    skip_group_check=True)
                        nc.vector.tensor_scalar_add(
                            qkT[:, j, ds(c0, cn)], ps[:, 0:cn],
                            bqk_sb[:, ds(j, 1)])
                    return th

                def v_pair(s_t, ns):
                    def th():
                        xb = xbuf[b]
                        ps = pjps.tile([P, 512], F32, name="pps")
                        for k in range(ns):
                            blk, half = (s_t + k) // 2, (s_t + k) % 2
                            for e_t in range(NT):
                                # start=True clears the whole PSUM bank, so
                                # only the very first matmul may set it.
                                nc.tensor.matmul(
                                    ps[:, ds(130 * k, 130)],
                                    lhsT=xb[:, blk, e_t, ds(half * P, P)],
                                    rhs=wv_sb[:, e_t, :],
                                    start=(k == 0 and e_t == 0),
                                    stop=(k == ns - 1 and e_t == NT - 1),
                                    skip_group_check=True)
                        nc.vector.tensor_tensor(
                            vp[:, ds(s_t, ns), :, :].rearrange(
                                "p s h d -> p (s h d)"),
                            ps[:, 0:130 * ns],
                            bvr_sb[:, 0:130 * ns],
                            mybir.AluOpType.add)
                    th.is_v = True
                    return th

                qk_list = [qk_chunk(*c) for c in qk_chunks(b)]
                v_list = [v_pair(s_t, min(2, T - s_t))
                          for s_t in range(0, T, 2)]
                thunks = []
                qi, vi = 0, 0
                while qi < len(qk_list) or vi < len(v_list):
                    if qi < len(qk_list):
                        thunks.append(qk_list[qi]); qi += 1
                    if vi < len(v_list):
                        thunks.append(v_list[vi]); vi += 1
                return thunks

            def emit_scores(b, t):
                """QK^T -> exp -> diag mask for k-tile t of batch b."""
                L = lpad[b]
                lq = lex[b]
                qkT, vp, eT, out_sb, toff = state[b]
                q0 = t * P
                qend = max(q0 + P, lq)
                chunks = [(c0, min(512, qend - c0))
                          for c0 in range(q0, qend, 512)]
                for ci, (c0, cn) in enumerate(chunks):
                    sps = sps_p.tile([P, 1024], F32, name="sps")
                    nc.tensor.matmul(
                        sps[:, 0:cn],
                        lhsT=qkT[0:DH, 1, ts(t, P)],
                        rhs=qkT[0:DH, 0, ds(c0, cn)],
                        start=True, stop=True)
                    nc.tensor.matmul(
                        sps[:, ds(512, cn)],
                        lhsT=qkT[DH:P, 1, ts(t, P)],
                        rhs=qkT[DH:P, 0, ds(c0, cn)],
                        start=True, stop=True)
                    nc.scalar.activation(
                        out=eT[:, :, ds(toff[t] + c0 - q0, cn)],
                        in_=sps.rearrange("p (h q) -> p h q", h=2)[
                            :, :, 0:cn],
                        func=mybir.ActivationFunctionType.Exp,
                        scale=1.0 / 32.0)
                    if ci == 0:
                        nc.vector.tensor_mul(
                            eT[:, :, ds(toff[t], P)],
                            eT[:, :, ds(toff[t], P)],
                            cm2_sb)

            def emit_attention(b, feeder, skip_t0, last):
                L = lpad[b]
                T = L // P
                qkT, vp, eT, out_sb, toff = state[b]
                nfeed0 = len(feeder)
                fed = 0

                def emit_out(t):
                    for h in range(2):
                        po = ops_p.tile([P, 512], F32, name="po")
                        for tk in range(t + 1):
                            nc.tensor.matmul(
                                po[:, 0:DH + 1],
                                lhsT=eT[:, h, ds(toff[tk] + (t - tk) * P, P)],
                                rhs=vp[:, tk, h, :],
                                start=(tk == 0),
                                stop=(tk == t))
                        nc.vector.tensor_copy(
                            out=out_sb[:, t, ds(h * 65, 65)],
                            in_=po[:, 0:DH + 1])

                for t in range(T):
                    if not (t == 0 and skip_t0):
                        emit_scores(b, t)
                    target = (t + 1) * nfeed0 // T
                    while fed < target and feeder:
                        feeder.popleft()(); fed += 1
                    emit_out(t)
                    if last:
                        nc.sync.dma_start(
                            out=o[:, ds((otile[b] + t) * 130, 130)],
                            in_=out_sb[:, t, :])
                if not last:
                    # one batch-sized DMA: per partition a single contiguous
                    # T*520B segment (per-row DMAs were descriptor-bound).
                    nc.sync.dma_start(
                        out=o[:, ds(otile[b] * 130, T * 130)],
                        in_=out_sb[:, 0:T, :].rearrange("p t d -> p (t d)"))
                while feeder:
                    feeder.popleft()()

            # ---- schedule ----
            for th in proj_thunks(order[0]):
                th()
            emit_scores(order[0], 0)
            for i, b in enumerate(order):
                feeder = deque()
                if i + 1 < B:
                    nb = order[i + 1]
                    feeder = deque(proj_thunks(nb))
                    feeder.append(lambda nb=nb: emit_scores(nb, 0))
                emit_attention(b, feeder, skip_t0=True, last=(i == B - 1))

    nc.compile()
    return nc


def _prepare_in_maps(x, l, W, b, lpad, l2, order, offs, stot2):
    W = np.asarray(W, dtype=np.float32)
    b = np.asarray(b, dtype=np.float32)

    # blocked x: xt2[p, blk, t, s] = x[batch, pos, e] with e = t*128+p,
    # pos = (blk - blk0_b)*256 + s
    nblk_tot = stot2 // BLK
    xt2 = np.empty((P, nblk_tot, NT, BLK), dtype=ml_dtypes.bfloat16)
    for bi in order:
        Lb2 = l2[bi]
        blk0 = offs[bi] // BLK
        # [L2, E] -> [nb2, 256, 8, 128] -> [128 p, nb2, 8 t, 256 s]
        xs = x[bi, :Lb2, :].reshape(Lb2 // BLK, BLK, NT, P).transpose(
            3, 0, 2, 1)
        xt2[:, blk0:blk0 + Lb2 // BLK] = xs.astype(ml_dtypes.bfloat16)
    xt2 = np.ascontiguousarray(xt2.reshape(P, nblk_tot * NT * BLK))

    k_idx = np.arange(P)[:, None]
    q_idx = np.arange(P)[None, :]
    cm = (k_idx <= q_idx).astype(np.float32)
    cm2 = np.concatenate([cm, cm], axis=1)  # [128, 256]

    in_maps = []
    for c in range(8):
        h0, h1 = 2 * c, 2 * c + 1
        rows_q = np.concatenate([
            W[h0 * DH:(h0 + 1) * DH],
            W[h1 * DH:(h1 + 1) * DH],
        ], axis=0)  # [128, E]
        rows_k = np.concatenate([
            W[E + h0 * DH:E + (h0 + 1) * DH],
            W[E + h1 * DH:E + (h1 + 1) * DH],
        ], axis=0)
        # [P part=e_in, t, i] = rows[i, t*128 + e_in]
        wq_c = np.ascontiguousarray(
            rows_q.T.reshape(NT, P, P).transpose(1, 0, 2)
            .reshape(P, NT * P).astype(ml_dtypes.bfloat16))
        wk_c = rows_k.T.reshape(NT, P, P).transpose(1, 0, 2).reshape(P, NT * P)
        wv_f = np.zeros((E, 130), dtype=np.float32)
        wv_f[:, 0:DH] = W[2 * E + h0 * DH:2 * E + (h0 + 1) * DH].T
        wv_f[:, DH + 1:2 * DH + 1] = W[2 * E + h1 * DH:2 * E + (h1 + 1) * DH].T
        wv_c = wv_f.reshape(NT, P, 130).transpose(1, 0, 2).reshape(P, NT * 130)
        constsB_c = np.ascontiguousarray(np.concatenate(
            [wk_c, wv_c, cm2], axis=1).astype(ml_dtypes.bfloat16))

        brows = np.concatenate([
            b[h0 * DH:(h0 + 1) * DH], b[h1 * DH:(h1 + 1) * DH],
            b[E + h0 * DH:E + (h0 + 1) * DH],
            b[E + h1 * DH:E + (h1 + 1) * DH],
        ])
        bqk_c = brows.reshape(2, P).T  # [128, 2] f32
        bvr = np.zeros((260,), dtype=np.float32)
        for k in range(2):
            bvr[130 * k + 0:130 * k + DH] = b[
                2 * E + h0 * DH:2 * E + (h0 + 1) * DH]
            bvr[130 * k + DH] = 1.0
            bvr[130 * k + DH + 1:130 * k + 2 * DH + 1] = b[
                2 * E + h1 * DH:2 * E + (h1 + 1) * DH]
            bvr[130 * k + 2 * DH + 1] = 1.0
        bconsts_c = np.ascontiguousarray(np.concatenate(
            [bqk_c, np.broadcast_to(bvr, (P, 260))], axis=1).astype(
                np.float32))
        in_maps.append({
            "xt2": xt2, "constsA": wq_c, "constsB": constsB_c,
            "bconsts": bconsts_c,
        })
    return in_maps


def _run(x, l, W, b, trace=False):
    x = np.asarray(x, dtype=np.float32)
    lv = np.asarray(l).astype(np.int64)
    lpad, l2, order, offs, stot2, lex = _plan(lv)
    key = (lpad, order, lex)
    if key not in _cached:
        _cached[key] = _build_program(lpad, l2, order, offs, stot2, lex)
    nc = _cached[key]
    in_maps = _prepare_in_maps(x, lv, W, b, lpad, l2, order, offs, stot2)
    res = run_bass_kernel_spmd(nc, in_maps, list(range(8)), trace=trace)
    out = np.zeros((B, S, E), dtype=np.float32)
    # output tile offsets in `order`
    otile = {}
    ot = 0
    for bi in order:
        otile[bi] = ot
        ot += lpad[bi] // P
    ntiles = ot
    for c in range(8):
        oc = np.asarray(res.results[c]["o"], dtype=np.float32)
        # [P, ntiles, 130] -> rows in q order per tile
        ocq = oc.reshape(P, ntiles, 130).transpose(1, 0, 2).reshape(
            ntiles * P, 130)
        for bi in range(B):
            n = int(min(lv[bi], lpad[bi]))
            blk = ocq[otile[bi] * P:otile[bi] * P + n, :]
            for h in range(2):
                den = blk[:, h * 65 + DH:h * 65 + DH + 1]
                out[bi, :n, (2 * c + h) * DH:(2 * c + h + 1) * DH] = (
                    blk[:, h * 65:h * 65 + DH] / den)
    return out, res.exec_time_ns


def kernel(x, l, W, b):
    out, _ = _run(x, l, W, b, trace=False)
    return out


# revision 8
# speedup vs baseline: 1.2213x; 1.2213x over previous
"""Multi-head self-attention (B=8, S=1024, E=1024, H=16) on 8 TRN2 cores.

Sharding: tensor-parallel on heads — core c owns heads (2c, 2c+1) for ALL
batches. Every core runs the identical program (SPMD); only the W/bias column
slices differ per core, so the load is balanced by construction.

Per-batch sequence truncation: batch b is processed only up to
L_b = ceil(l_b/128)*128 rows (compile-time constants baked per call, cached on
the value of l). Rows q >= l_b are discarded on the host; causal masking makes
the padded key columns inside the last tile harmless for valid rows.

x lives in HBM in a blocked layout [128 part][blk][e_t][256] so that any
span of 256-blocks is one DMA with 128 large contiguous descriptors (the
per-row descriptor storm of the naive [E, stot] layout was both dispatch-
and queue-bound).

Engine plan per core:
  PE : QKV projection (contraction over E in 8 tiles of 128), scores QK^T
       (contraction 64 -> two PE row-halves in parallel), out = E@V with the
       softmax denominator as a ones column of V.
  ACT: exp only (one table set, loaded once at start), two heads per
       ACTIVATE instruction to amortize the fixed overhead.
  DVE: PSUM->SBUF copies (QK bias-add, paired V tiles with the V-bias fused
       into the copy, out rows), causal diag mask per k-tile.
  Out rows carry the denominator (65 cols per head); division on the host.
Emission software-pipelines proj(b+1) and the first score tile of (b+1) into
attention(b) to cover ACT latency and batch transitions.
"""

import sys

sys.path.insert(0, "/opt/trn_rl_repo")

from collections import deque

import numpy as np
import ml_dtypes

import concourse.bass as bass
import concourse.bacc as bacc
import concourse.mybir as mybir
import concourse.tile as tile
from concourse.bass import ds, ts
from concourse.bass_utils import run_bass_kernel_spmd

P = 128
BLK = 256
B, S, E, H = 8, 1024, 1024, 16
DH = E // H  # 64
F32 = mybir.dt.float32
BF16 = mybir.dt.bfloat16
ETDT = BF16
NT = E // P  # 8 contraction tiles

_cached = {}


def _plan(l):
    lex = [min(int(v), S) for v in l]
    lpad = [min((int(v) + P - 1) // P * P, S) for v in l]
    l2 = [(Lp + BLK - 1) // BLK * BLK for Lp in lpad]
    order = sorted(range(B), key=lambda b: -lpad[b])
    offs = {}   # in positions, within the padded (l2) layout
    off = 0
    for b in order:
        offs[b] = off
        off += l2[b]
    return tuple(lpad), tuple(l2), tuple(order), offs, off, tuple(lex)


def _build_program(lpad, l2, order, offs, stot2, lex):
    nc = bacc.Bacc(None, target_bir_lowering=False)

    # x: [128][blk][e_t][256] blocked layout (see module docstring)
    xt2 = nc.dram_tensor(
        "xt2", [P, (stot2 // BLK) * NT * BLK], BF16, kind="ExternalInput")[:]
    # constsA: wqk Q columns [P, NT, 128]
    constsA = nc.dram_tensor("constsA", [P, NT * P], BF16,
                             kind="ExternalInput")[:]
    # constsB: wqk K columns [P, NT*128] | wv [P, NT*130] | cm2 [P, 2*128]
    CB_W = NT * P + NT * 130 + 2 * P
    constsB = nc.dram_tensor("constsB", [P, CB_W], BF16,
                             kind="ExternalInput")[:]
    # bconsts: f32: cols 0:2 = qk bias (q, k); 2:262 = V bias row
    # replicated across partitions (with 1.0 denominator columns)
    bconsts = nc.dram_tensor("bconsts", [P, 262], F32, kind="ExternalInput")[:]
    # out: per batch tile t: [P, 130] (two heads x (64 + denom))
    o = nc.dram_tensor("o", [P, sum(Lp // P for Lp in lpad) * 130], F32,
                       kind="ExternalOutput")[:]
    # output tile offsets (in 130-col units) per batch, in `order`
    otile = {}
    ot = 0
    for b in order:
        otile[b] = ot
        ot += lpad[b] // P

    with tile.TileContext(nc) as tc:
        from contextlib import ExitStack

        with ExitStack() as ctx:
            sb = ctx.enter_context(tc.tile_pool(name="sb", bufs=1))
            wqkq_sb = sb.tile([P, NT, P], BF16)
            constsB_sb = sb.tile([P, CB_W], BF16)
            wqkk_sb = constsB_sb[:, 0:NT * P].rearrange(
                "p (t i) -> p t i", t=NT)
            wv_sb = constsB_sb[:, ds(NT * P, NT * 130)].rearrange(
                "p (t i) -> p t i", t=NT)
            cm2_sb = constsB_sb[:, ds(NT * P + NT * 130, 2 * P)].rearrange(
                "p (h q) -> p h q", h=2)
            bconsts_sb = sb.tile([P, 262], F32)
            bqk_sb = bconsts_sb[:, 0:2]
            bvr_sb = bconsts_sb[:, 2:262]
            warm_sb = sb.tile([P, 8], F32)
            warm2_sb = sb.tile([P, 8], BF16)

            xp = ctx.enter_context(tc.tile_pool(name="xp", bufs=1))
            qkp = ctx.enter_context(tc.tile_pool(name="qkp", bufs=2))
            vpp = ctx.enter_context(tc.tile_pool(name="vpp", bufs=2))
            eTp = ctx.enter_context(tc.tile_pool(name="eTp", bufs=2))
            outp = ctx.enter_context(tc.tile_pool(name="outp", bufs=4))
            pjps = ctx.enter_context(
                tc.tile_pool(name="pjps", bufs=2, space="PSUM"))
            sps_p = ctx.enter_context(
                tc.tile_pool(name="sps", bufs=2, space="PSUM"))
            ops_p = ctx.enter_context(
                tc.tile_pool(name="ops", bufs=2, space="PSUM"))

            # Warm the ACT exp table ASAP (overlaps const DMAs + first x DMA).
            nc.vector.memset(warm_sb, 0.0)
            nc.scalar.activation(
                out=warm2_sb, in_=warm_sb,
                func=mybir.ActivationFunctionType.Exp, scale=1.0)

            xbuf = {}

            def emit_xdma(b, eng, spans):
                """spans: list of (blk0, nblk) in 256-block units."""
                nb2 = l2[b] // BLK
                if b not in xbuf:
                    xbuf[b] = xp.tile([P, nb2, NT, BLK], BF16,
                                      name=f"xb{b}", tag=f"xb{b}")
                xb = xbuf[b]
                xsrc = xt2.rearrange("p (k t s) -> p k t s", t=NT, s=BLK)
                base = offs[b] // BLK
                for (a, n) in spans:
                    eng.dma_start(
                        out=xb[:, ds(a, n), :, :],
                        in_=xsrc[:, ds(base + a, n), :, :])

            # ---- DMA schedule ----
            # Both hardware DGE engines dispatch (sync=SP, scalar=ACT);
            # gpsimd swdge is slow and contends on the same queues - avoid.
            b0 = order[0]
            nb2_0 = l2[b0] // BLK
            emit_xdma(b0, nc.sync, [(0, min(2, nb2_0))])
            nc.sync.dma_start(
                out=wqkq_sb, in_=constsA.rearrange("p (t i) -> p t i", t=NT))
            nc.sync.dma_start(out=constsB_sb, in_=constsB)
            if nb2_0 > 2:
                emit_xdma(b0, nc.sync, [(2, nb2_0 - 2)])
            nc.sync.dma_start(out=bconsts_sb, in_=bconsts)
            for i, b in enumerate(order[1:]):
                eng = nc.sync if i % 2 == 0 else nc.scalar
                emit_xdma(b, eng, [(0, l2[b] // BLK)])

            state = {}

            def qk_chunks(b):
                """(j, c0, cn) list; block-aligned starts, 512 preferred."""
                lq = lex[b]
                out = []
                for j, end in ((0, lq), (1, lpad[b])):
                    c0 = 0
                    while c0 < end:
                        cn = min(512, end - c0)
                        out.append((j, c0, cn))
                        c0 += cn
                return out

            def proj_thunks(b):
                """Closures emitting proj(b); each ~0.5-2us of PE work."""
                L = lpad[b]
                T = L // P
                qkT = qkp.tile([P, 2, L], BF16, name="qkT")
                vp = vpp.tile([P, T, 2, DH + 1], BF16, name="vp")
                toff = [t * L - t * (t - 1) * P // 2 for t in range(T + 1)]
                eT = eTp.tile([P, 2, toff[T]], ETDT, name="eT")
                out_sb = outp.tile([P, T, 130], F32, name="out_sb")
                state[b] = (qkT, vp, eT, out_sb, toff)

                def qk_chunk(j, c0, cn):
                    def th():
                        xb = xbuf[b]
                        ps = pjps.tile([P, 512], F32, name="pps")
                        # moving operand: whole blocks, then ragged tail
                        nwhole = (cn // BLK) * BLK
                        for e_t in range(NT):
                            if nwhole:
                                nc.tensor.matmul(
                                    ps[:, 0:nwhole],
                                    lhsT=(wqkq_sb if j == 0 else wqkk_sb)[
                                        :, e_t, :],
                                    rhs=xb[:, ds(c0 // BLK, nwhole // BLK),
                                           e_t, :],
                                    start=(e_t == 0),
                                    stop=(e_t == NT - 1 and cn == nwhole))
                        for e_t in range(NT):
                            if cn > nwhole:
                                nc.tensor.matmul(
                                    ps[:, ds(nwhole, cn - nwhole)],
                                    lhsT=(wqkq_sb if j == 0 else wqkk_sb)[
                                        :, e_t, :],
                                    rhs=xb[:, (c0 + nwhole) // BLK, e_t,
                                           ds(0, cn - nwhole)],
                                    start=(e_t == 0 and nwhole == 0),
                                    stop=(e_t == NT - 1),
                                    skip_group_check=True)
                        nc.vector.tensor_scalar_add(
                            qkT[:, j, ds(c0, cn)], ps[:, 0:cn],
                            bqk_sb[:, ds(j, 1)])
                    return th

                def v_pair(s_t, ns):
                    def th():
                        xb = xbuf[b]
                        ps = pjps.tile([P, 512], F32, name="pps")
                        for k in range(ns):
                            blk, half = (s_t + k) // 2, (s_t + k) % 2
                            for e_t in range(NT):
                                # start=True clears the whole PSUM bank, so
                                # only the very first matmul may set it.
                                nc.tensor.matmul(
                                    ps[:, ds(130 * k, 130)],
                                    lhsT=xb[:, blk, e_t, ds(half * P, P)],
                                    rhs=wv_sb[:, e_t, :],
                                    start=(k == 0 and e_t == 0),
                                    stop=(k == ns - 1 and e_t == NT - 1),
                                    skip_group_check=True)
                        nc.vector.tensor_tensor(
                            vp[:, ds(s_t, ns), :, :].rearrange(
                                "p s h d -> p (s h d)"),
                            ps[:, 0:130 * ns],
                            bvr_sb[:, 0:130 * ns],
                            mybir.AluOpType.add)
                    th.is_v = True
                    return th

                qk_list = [qk_chunk(*c) for c in qk_chunks(b)]
                v_list = [v_pair(s_t, min(2, T - s_t))
                          for s_t in range(0, T, 2)]
                thunks = []
                qi, vi = 0, 0
                while qi < len(qk_list) or vi < len(v_list):
                    if qi < len(qk_list):
                        thunks.append(qk_list[qi]); qi += 1
                    if vi < len(v_list):
                        thunks.append(v_list[vi]); vi += 1
                return thunks

            def emit_scores(b, t):
                """QK^T -> exp -> diag mask for k-tile t of batch b."""
                L = lpad[b]
                lq = lex[b]
                qkT, vp, eT, out_sb, toff = state[b]
                q0 = t * P
                qend = max(q0 + P, lq)
                chunks = [(c0, min(512, qend - c0))
                          for c0 in range(q0, qend, 512)]
                for ci, (c0, cn) in enumerate(chunks):
                    sps = sps_p.tile([P, 1024], F32, name="sps")
                    nc.tensor.matmul(
                        sps[:, 0:cn],
                        lhsT=qkT[0:DH, 1, ts(t, P)],
                        rhs=qkT[0:DH, 0, ds(c0, cn)],
                        start=True, stop=True)
                    nc.tensor.matmul(
                        sps[:, ds(512, cn)],
                        lhsT=qkT[DH:P, 1, ts(t, P)],
                        rhs=qkT[DH:P, 0, ds(c0, cn)],
                        start=True, stop=True)
                    nc.scalar.activation(
                        out=eT[:, :, ds(toff[t] + c0 - q0, cn)],
                        in_=sps.rearrange("p (h q) -> p h q", h=2)[
                            :, :, 0:cn],
                        func=mybir.ActivationFunctionType.Exp,
                        scale=1.0 / 32.0)
                    if ci == 0:
                        nc.vector.tensor_mul(
                            eT[:, :, ds(toff[t], P)],
                            eT[:, :, ds(toff[t], P)],
                            cm2_sb)

            def emit_attention(b, feeder, skip_t0, last):
                L = lpad[b]
                T = L // P
                qkT, vp, eT, out_sb, toff = state[b]
                nfeed0 = len(feeder)
                fed = 0

                def emit_out(t):
                    for h in range(2):
                        po = ops_p.tile([P, 512], F32, name="po")
                        for tk in range(t + 1):
                            nc.tensor.matmul(
                                po[:, 0:DH + 1],
                                lhsT=eT[:, h, ds(toff[tk] + (t - tk) * P, P)],
                                rhs=vp[:, tk, h, :],
                                start=(tk == 0),
                                stop=(tk == t))
                        nc.vector.tensor_copy(
                            out=out_sb[:, t, ds(h * 65, 65)],
                            in_=po[:, 0:DH + 1])

                for t in range(T):
                    if not (t == 0 and skip_t0):
                        emit_scores(b, t)
                    target = (t + 1) * nfeed0 // T
                    while fed < target and feeder:
                        feeder.popleft()(); fed += 1
                    emit_out(t)
                    if last:
                        nc.sync.dma_start(
                            out=o[:, ds((otile[b] + t) * 130, 130)],
                            in_=out_sb[:, t, :])
                if not last:
                    # one batch-sized DMA: per partition a single contiguous
                    # T*520B segment (per-row DMAs were descriptor-bound).
                    nc.sync.dma_start(
                        out=o[:, ds(otile[b] * 130, T * 130)],
                        in_=out_sb[:, 0:T, :].rearrange("p t d -> p (t d)"))
                while feeder:
                    feeder.popleft()()

            # ---- schedule ----
            for th in proj_thunks(order[0]):
                th()
            emit_scores(order[0], 0)
            for i, b in enumerate(order):
                feeder = deque()
                if i + 1 < B:
                    nb = order[i + 1]
                    feeder = deque(proj_thunks(nb))
                    feeder.append(lambda nb=nb: emit_scores(nb, 0))
                emit_attention(b, feeder, skip_t0=True, last=(i == B - 1))

    nc.compile()
    return nc


def _prepare_in_maps(x, l, W, b, lpad, l2, order, offs, stot2):
    W = np.asarray(W, dtype=np.float32)
    b = np.asarray(b, dtype=np.float32)

    # blocked x: xt2[p, blk, t, s] = x[batch, pos, e] with e = t*128+p,
    # pos = (blk - blk0_b)*256 + s
    nblk_tot = stot2 // BLK
    xt2 = np.empty((P, nblk_tot, NT, BLK), dtype=ml_dtypes.bfloat16)
    for bi in order:
        Lb2 = l2[bi]
        blk0 = offs[bi] // BLK
        # [L2, E] -> [nb2, 256, 8, 128] -> [128 p, nb2, 8 t, 256 s]
        xs = x[bi, :Lb2, :].reshape(Lb2 // BLK, BLK, NT, P).transpose(
            3, 0, 2, 1)
        xt2[:, blk0:blk0 + Lb2 // BLK] = xs.astype(ml_dtypes.bfloat16)
    xt2 = np.ascontiguousarray(xt2.reshape(P, nblk_tot * NT * BLK))

    k_idx = np.arange(P)[:, None]
    q_idx = np.arange(P)[None, :]
    cm = (k_idx <= q_idx).astype(np.float32)
    cm2 = np.concatenate([cm, cm], axis=1)  # [128, 256]

    in_maps = []
    for c in range(8):
        h0, h1 = 2 * c, 2 * c + 1
        rows_q = np.concatenate([
            W[h0 * DH:(h0 + 1) * DH],
            W[h1 * DH:(h1 + 1) * DH],
        ], axis=0)  # [128, E]
        rows_k = np.concatenate([
            W[E + h0 * DH:E + (h0 + 1) * DH],
            W[E + h1 * DH:E + (h1 + 1) * DH],
        ], axis=0)
        # [P part=e_in, t, i] = rows[i, t*128 + e_in]
        wq_c = np.ascontiguousarray(
            rows_q.T.reshape(NT, P, P).transpose(1, 0, 2)
            .reshape(P, NT * P).astype(ml_dtypes.bfloat16))
        wk_c = rows_k.T.reshape(NT, P, P).transpose(1, 0, 2).reshape(P, NT * P)
        wv_f = np.zeros((E, 130), dtype=np.float32)
        wv_f[:, 0:DH] = W[2 * E + h0 * DH:2 * E + (h0 + 1) * DH].T
        wv_f[:, DH + 1:2 * DH + 1] = W[2 * E + h1 * DH:2 * E + (h1 + 1) * DH].T
        wv_c = wv_f.reshape(NT, P, 130).transpose(1, 0, 2).reshape(P, NT * 130)
        constsB_c = np.ascontiguousarray(np.concatenate(
            [wk_c, wv_c, cm2], axis=1).astype(ml_dtypes.bfloat16))

        brows = np.concatenate([
            b[h0 * DH:(h0 + 1) * DH], b[h1 * DH:(h1 + 1) * DH],
            b[E + h0 * DH:E + (h0 + 1) * DH],
            b[E + h1 * DH:E + (h1 + 1) * DH],
        ])
        bqk_c = brows.reshape(2, P).T  # [128, 2] f32
        bvr = np.zeros((260,), dtype=np.float32)
        for k in range(2):
            bvr[130 * k + 0:130 * k + DH] = b[
                2 * E + h0 * DH:2 * E + (h0 + 1) * DH]
            bvr[130 * k + DH] = 1.0
            bvr[130 * k + DH + 1:130 * k + 2 * DH + 1] = b[
                2 * E + h1 * DH:2 * E + (h1 + 1) * DH]
            bvr[130 * k + 2 * DH + 1] = 1.0
        bconsts_c = np.ascontiguousarray(np.concatenate(
            [bqk_c, np.broadcast_to(bvr, (P, 260))], axis=1).astype(
                np.float32))
        in_maps.append({
            "xt2": xt2, "constsA": wq_c, "constsB": constsB_c,
            "bconsts": bconsts_c,
        })
    return in_maps


def _run(x, l, W, b, trace=False):
    x = np.asarray(x, dtype=np.float32)
    lv = np.asarray(l).astype(np.int64)
    lpad, l2, order, offs, stot2, lex = _plan(lv)
    key = (lpad, order, lex)
    if key not in _cached:
        _cached[key] = _build_program(lpad, l2, order, offs, stot2, lex)
    nc = _cached[key]
    in_maps = _prepare_in_maps(x, lv, W, b, lpad, l2, order, offs, stot2)
    res = run_bass_kernel_spmd(nc, in_maps, list(range(8)), trace=trace)
    out = np.zeros((B, S, E), dtype=np.float32)
    # output tile offsets in `order`
    otile = {}
    ot = 0
    for bi in order:
        otile[bi] = ot
        ot += lpad[bi] // P
    ntiles = ot
    for c in range(8):
        oc = np.asarray(res.results[c]["o"], dtype=np.float32)
        # [P, ntiles, 130] -> rows in q order per tile
        ocq = oc.reshape(P, ntiles, 130).transpose(1, 0, 2).reshape(
            ntiles * P, 130)
        for bi in range(B):
            n = int(min(lv[bi], lpad[bi]))
            blk = ocq[otile[bi] * P:otile[bi] * P + n, :]
            for h in range(2):
                den = blk[:, h * 65 + DH:h * 65 + DH + 1]
                out[bi, :n, (2 * c + h) * DH:(2 * c + h + 1) * DH] = (
                    blk[:, h * 65:h * 65 + DH] / den)
    return out, res.exec_time_ns


def kernel(x, l, W, b):
    out, _ = _run(x, l, W, b, trace=False)
    return out


# revision 9
# speedup vs baseline: 1.3241x; 1.0842x over previous
"""Multi-head self-attention (B=8, S=1024, E=1024, H=16) on 8 TRN2 cores.

Sharding: tensor-parallel on heads — core c owns heads (2c, 2c+1) for ALL
batches. Every core runs the identical program (SPMD); only the W/bias column
slices differ per core, so the load is balanced by construction.

Per-batch sequence truncation: batch b is processed only up to
L_b = ceil(l_b/128)*128 rows (compile-time constants baked per call, cached on
the value of l). Rows q >= l_b are discarded on the host; causal masking makes
the padded key columns inside the last tile harmless for valid rows.

x lives in HBM in a blocked layout [128 part][blk][e_t][256] so that any
span of 256-blocks is one DMA with 128 large contiguous descriptors (the
per-row descriptor storm of the naive [E, stot] layout was both dispatch-
and queue-bound).

Engine plan per core:
  PE : QKV projection (contraction over E in 8 tiles of 128), scores QK^T
       (contraction 64 -> two PE row-halves in parallel), out = E@V with the
       softmax denominator as a ones column of V.
  ACT: exp only (one table set, loaded once at start), two heads per
       ACTIVATE instruction to amortize the fixed overhead.
  DVE: PSUM->SBUF copies (QK bias-add, paired V tiles with the V-bias fused
       into the copy, out rows), causal diag mask per k-tile.
  Out rows carry the denominator (65 cols per head); division on the host.
Emission software-pipelines proj(b+1) and the first score tile of (b+1) into
attention(b) to cover ACT latency and batch transitions.
"""

import sys

sys.path.insert(0, "/opt/trn_rl_repo")

from collections import deque

import numpy as np
import ml_dtypes

import concourse.bass as bass
import concourse.bacc as bacc
import concourse.mybir as mybir
import concourse.tile as tile
from concourse.bass import ds, ts
from concourse.bass_utils import run_bass_kernel_spmd

P = 128
BLK = 256
B, S, E, H = 8, 1024, 1024, 16
DH = E // H  # 64
F32 = mybir.dt.float32
BF16 = mybir.dt.bfloat16
ETDT = BF16
NT = E // P  # 8 contraction tiles

_cached = {}


def _plan(l):
    lex = [min(int(v), S) for v in l]
    lpad = [min((int(v) + P - 1) // P * P, S) for v in l]
    l2 = [(Lp + BLK - 1) // BLK * BLK for Lp in lpad]
    order = sorted(range(B), key=lambda b: -lpad[b])
    offs = {}   # in positions, within the padded (l2) layout
    off = 0
    for b in order:
        offs[b] = off
        off += l2[b]
    return tuple(lpad), tuple(l2), tuple(order), offs, off, tuple(lex)


def _build_program(lpad, l2, order, offs, stot2, lex):
    nc = bacc.Bacc(None, target_bir_lowering=False)

    # x: [128][blk][e_t][256] blocked layout (see module docstring)
    xt2 = nc.dram_tensor(
        "xt2", [P, (stot2 // BLK) * NT * BLK], BF16, kind="ExternalInput")[:]
    # constsA: wqk Q columns [P, NT, 128]
    constsA = nc.dram_tensor("constsA", [P, NT * P], BF16,
                             kind="ExternalInput")[:]
    # constsB: wqk K columns [P, NT*128] | wv [P, NT*130] | cm2 [P, 2*128]
    CB_W = NT * P + NT * 130 + 2 * P
    constsB = nc.dram_tensor("constsB", [P, CB_W], BF16,
                             kind="ExternalInput")[:]
    # bconsts: f32: cols 0:2 = qk bias (q, k); 2:262 = V bias row
    # replicated across partitions (with 1.0 denominator columns)
    bconsts = nc.dram_tensor("bconsts", [P, 262], F32, kind="ExternalInput")[:]
    # out: per batch tile t: [P, 130] (two heads x (64 + denom))
    o = nc.dram_tensor("o", [P, sum(Lp // P for Lp in lpad) * 130], F32,
                       kind="ExternalOutput")[:]
    # output tile offsets (in 130-col units) per batch, in `order`
    otile = {}
    ot = 0
    for b in order:
        otile[b] = ot
        ot += lpad[b] // P

    with tile.TileContext(nc) as tc:
        from contextlib import ExitStack

        with ExitStack() as ctx:
            sb = ctx.enter_context(tc.tile_pool(name="sb", bufs=1))
            wqkq_sb = sb.tile([P, NT, P], BF16)
            constsB_sb = sb.tile([P, CB_W], BF16)
            wqkk_sb = constsB_sb[:, 0:NT * P].rearrange(
                "p (t i) -> p t i", t=NT)
            wv_sb = constsB_sb[:, ds(NT * P, NT * 130)].rearrange(
                "p (t i) -> p t i", t=NT)
            cm2_sb = constsB_sb[:, ds(NT * P + NT * 130, 2 * P)].rearrange(
                "p (h q) -> p h q", h=2)
            bconsts_sb = sb.tile([P, 262], F32)
            bqk_sb = bconsts_sb[:, 0:2]
            bvr_sb = bconsts_sb[:, 2:262]
            warm_sb = sb.tile([P, 8], F32)
            warm2_sb = sb.tile([P, 8], BF16)

            xp = ctx.enter_context(tc.tile_pool(name="xp", bufs=1))
            qkp = ctx.enter_context(tc.tile_pool(name="qkp", bufs=2))
            vpp = ctx.enter_context(tc.tile_pool(name="vpp", bufs=2))
            eTp = ctx.enter_context(tc.tile_pool(name="eTp", bufs=2))
            outp = ctx.enter_context(tc.tile_pool(name="outp", bufs=4))
            pjps = ctx.enter_context(
                tc.tile_pool(name="pjps", bufs=2, space="PSUM"))
            sps_p = ctx.enter_context(
                tc.tile_pool(name="sps", bufs=2, space="PSUM"))
            ops_p = ctx.enter_context(
                tc.tile_pool(name="ops", bufs=2, space="PSUM"))

            # Warm the ACT exp table ASAP (overlaps const DMAs + first x DMA).
            nc.vector.memset(warm_sb, 0.0)
            nc.scalar.activation(
                out=warm2_sb, in_=warm_sb,
                func=mybir.ActivationFunctionType.Exp, scale=1.0)

            xbuf = {}

            def emit_xdma(b, eng, spans):
                """spans: list of (blk0, nblk) in 256-block units."""
                nb2 = l2[b] // BLK
                if b not in xbuf:
                    xbuf[b] = xp.tile([P, nb2, NT, BLK], BF16,
                                      name=f"xb{b}", tag=f"xb{b}")
                xb = xbuf[b]
                xsrc = xt2.rearrange("p (k t s) -> p k t s", t=NT, s=BLK)
                base = offs[b] // BLK
                for (a, n) in spans:
                    eng.dma_start(
                        out=xb[:, ds(a, n), :, :],
                        in_=xsrc[:, ds(base + a, n), :, :])

            # ---- DMA schedule ----
            # Consumers wait on the CUMULATIVE DMA stream of the dispatching
            # engine, so dispatch strictly in consumer order. Both hardware
            # DGE engines dispatch (sync=SP, scalar=ACT); gpsimd swdge is
            # slow and contends on the same queues - avoid.
            b0 = order[0]
            nb2_0 = l2[b0] // BLK
            nc.sync.dma_start(
                out=wqkq_sb, in_=constsA.rearrange("p (t i) -> p t i", t=NT))
            emit_xdma(b0, nc.sync, [(0, min(2, nb2_0))])
            nc.sync.dma_start(out=bconsts_sb, in_=bconsts)
            nc.sync.dma_start(out=constsB_sb, in_=constsB)
            if nb2_0 > 2:
                emit_xdma(b0, nc.sync, [(2, nb2_0 - 2)])
            for i, b in enumerate(order[1:]):
                eng = nc.sync if i % 2 == 0 else nc.scalar
                emit_xdma(b, eng, [(0, l2[b] // BLK)])

            state = {}

            def qk_chunks(b):
                """(j, c0, cn) list; block-aligned starts, 512 preferred."""
                lq = lex[b]
                out = []
                for j, end in ((0, lq), (1, lpad[b])):
                    c0 = 0
                    while c0 < end:
                        cn = min(512, end - c0)
                        out.append((j, c0, cn))
                        c0 += cn
                return out

            def proj_thunks(b):
                """Closures emitting proj(b); each ~0.5-2us of PE work."""
                L = lpad[b]
                T = L // P
                qkT = qkp.tile([P, 2, L], BF16, name="qkT")
                vp = vpp.tile([P, T, 2, DH + 1], BF16, name="vp")
                toff = [t * L - t * (t - 1) * P // 2 for t in range(T + 1)]
                eT = eTp.tile([P, 2, toff[T]], ETDT, name="eT")
                out_sb = outp.tile([P, T, 130], F32, name="out_sb")
                state[b] = (qkT, vp, eT, out_sb, toff)

                def qk_chunk(j, c0, cn):
                    def th():
                        xb = xbuf[b]
                        ps = pjps.tile([P, 512], F32, name="pps")
                        # moving operand: whole blocks, then ragged tail
                        nwhole = (cn // BLK) * BLK
                        for e_t in range(NT):
                            if nwhole:
                                nc.tensor.matmul(
                                    ps[:, 0:nwhole],
                                    lhsT=(wqkq_sb if j == 0 else wqkk_sb)[
                                        :, e_t, :],
                                    rhs=xb[:, ds(c0 // BLK, nwhole // BLK),
                                           e_t, :],
                                    start=(e_t == 0),
                                    stop=(e_t == NT - 1 and cn == nwhole))
                        for e_t in range(NT):
                            if cn > nwhole:
                                nc.tensor.matmul(
                                    ps[:, ds(nwhole, cn - nwhole)],
                                    lhsT=(wqkq_sb if j == 0 else wqkk_sb)[
                                        :, e_t, :],
                                    rhs=xb[:, (c0 + nwhole) // BLK, e_t,
                                           ds(0, cn - nwhole)],
                                    start=(e_t == 0 and nwhole == 0),
                                    stop=(e_t == NT - 1),
                                    skip_group_check=True)
                        nc.vector.tensor_scalar_add(
                            qkT[:, j, ds(c0, cn)], ps[:, 0:cn],
                            bqk_sb[:, ds(j, 1)])
                    return th

                def v_pair(s_t, ns):
                    def th():
                        xb = xbuf[b]
                        ps = pjps.tile([P, 512], F32, name="pps")
                        for k in range(ns):
                            blk, half = (s_t + k) // 2, (s_t + k) % 2
                            for e_t in range(NT):
                                # start=True clears the whole PSUM bank, so
                                # only the very first matmul may set it.
                                nc.tensor.matmul(
                                    ps[:, ds(130 * k, 130)],
                                    lhsT=xb[:, blk, e_t, ds(half * P, P)],
                                    rhs=wv_sb[:, e_t, :],
                                    start=(k == 0 and e_t == 0),
                                    stop=(k == ns - 1 and e_t == NT - 1),
                                    skip_group_check=True)
                        nc.vector.tensor_tensor(
                            vp[:, ds(s_t, ns), :, :].rearrange(
                                "p s h d -> p (s h d)"),
                            ps[:, 0:130 * ns],
                            bvr_sb[:, 0:130 * ns],
                            mybir.AluOpType.add)
                    th.is_v = True
                    return th

                qk_list = [qk_chunk(*c) for c in qk_chunks(b)]
                v_list = [v_pair(s_t, min(2, T - s_t))
                          for s_t in range(0, T, 2)]
                thunks = []
                qi, vi = 0, 0
                while qi < len(qk_list) or vi < len(v_list):
                    if qi < len(qk_list):
                        thunks.append(qk_list[qi]); qi += 1
                    if vi < len(v_list):
                        thunks.append(v_list[vi]); vi += 1
                return thunks

            def emit_scores(b, t):
                """QK^T -> exp -> diag mask for k-tile t of batch b."""
                L = lpad[b]
                lq = lex[b]
                qkT, vp, eT, out_sb, toff = state[b]
                q0 = t * P
                qend = max(q0 + P, lq)
                chunks = [(c0, min(512, qend - c0))
                          for c0 in range(q0, qend, 512)]
                for ci, (c0, cn) in enumerate(chunks):
                    sps = sps_p.tile([P, 1024], F32, name="sps")
                    nc.tensor.matmul(
                        sps[:, 0:cn],
                        lhsT=qkT[0:DH, 1, ts(t, P)],
                        rhs=qkT[0:DH, 0, ds(c0, cn)],
                        start=True, stop=True)
                    nc.tensor.matmul(
                        sps[:, ds(512, cn)],
                        lhsT=qkT[DH:P, 1, ts(t, P)],
                        rhs=qkT[DH:P, 0, ds(c0, cn)],
                        start=True, stop=True)
                    nc.scalar.activation(
                        out=eT[:, :, ds(toff[t] + c0 - q0, cn)],
                        in_=sps.rearrange("p (h q) -> p h q", h=2)[
                            :, :, 0:cn],
                        func=mybir.ActivationFunctionType.Exp,
                        scale=1.0 / 32.0)
                    if ci == 0:
                        nc.vector.tensor_mul(
                            eT[:, :, ds(toff[t], P)],
                            eT[:, :, ds(toff[t], P)],
                            cm2_sb)

            def emit_attention(b, feeder, skip_t0, last):
                L = lpad[b]
                T = L // P
                qkT, vp, eT, out_sb, toff = state[b]
                nfeed0 = len(feeder)
                fed = 0

                def emit_out(t):
                    for h in range(2):
                        po = ops_p.tile([P, 512], F32, name="po")
                        for tk in range(t + 1):
                            nc.tensor.matmul(
                                po[:, 0:DH + 1],
                                lhsT=eT[:, h, ds(toff[tk] + (t - tk) * P, P)],
                                rhs=vp[:, tk, h, :],
                                start=(tk == 0),
                                stop=(tk == t))
                        nc.vector.tensor_copy(
                            out=out_sb[:, t, ds(h * 65, 65)],
                            in_=po[:, 0:DH + 1])

                for t in range(T):
                    if not (t == 0 and skip_t0):
                        emit_scores(b, t)
                    target = (t + 1) * nfeed0 // T
                    while fed < target and feeder:
                        feeder.popleft()(); fed += 1
                    emit_out(t)
                    if last:
                        nc.sync.dma_start(
                            out=o[:, ds((otile[b] + t) * 130, 130)],
                            in_=out_sb[:, t, :])
                if not last:
                    # one batch-sized DMA: per partition a single contiguous
                    # T*520B segment (per-row DMAs were descriptor-bound).
                    nc.sync.dma_start(
                        out=o[:, ds(otile[b] * 130, T * 130)],
                        in_=out_sb[:, 0:T, :].rearrange("p t d -> p (t d)"))
                while feeder:
                    feeder.popleft()()

            # ---- schedule ----
            for th in proj_thunks(order[0]):
                th()
            emit_scores(order[0], 0)
            for i, b in enumerate(order):
                feeder = deque()
                if i + 1 < B:
                    nb = order[i + 1]
                    feeder = deque(proj_thunks(nb))
                    feeder.append(lambda nb=nb: emit_scores(nb, 0))
                emit_attention(b, feeder, skip_t0=True, last=(i == B - 1))

    nc.compile()
    return nc


def _prepare_in_maps(x, l, W, b, lpad, l2, order, offs, stot2):
    W = np.asarray(W, dtype=np.float32)
    b = np.asarray(b, dtype=np.float32)

    # blocked x: xt2[p, blk, t, s] = x[batch, pos, e] with e = t*128+p,
    # pos = (blk - blk0_b)*256 + s
    nblk_tot = stot2 // BLK
    xt2 = np.empty((P, nblk_tot, NT, BLK), dtype=ml_dtypes.bfloat16)
    for bi in order:
        Lb2 = l2[bi]
        blk0 = offs[bi] // BLK
        # [L2, E] -> [nb2, 256, 8, 128] -> [128 p, nb2, 8 t, 256 s]
        xs = x[bi, :Lb2, :].reshape(Lb2 // BLK, BLK, NT, P).transpose(
            3, 0, 2, 1)
        xt2[:, blk0:blk0 + Lb2 // BLK] = xs.astype(ml_dtypes.bfloat16)
    xt2 = np.ascontiguousarray(xt2.reshape(P, nblk_tot * NT * BLK))

    k_idx = np.arange(P)[:, None]
    q_idx = np.arange(P)[None, :]
    cm = (k_idx <= q_idx).astype(np.float32)
    cm2 = np.concatenate([cm, cm], axis=1)  # [128, 256]

    in_maps = []
    for c in range(8):
        h0, h1 = 2 * c, 2 * c + 1
        rows_q = np.concatenate([
            W[h0 * DH:(h0 + 1) * DH],
            W[h1 * DH:(h1 + 1) * DH],
        ], axis=0)  # [128, E]
        rows_k = np.concatenate([
            W[E + h0 * DH:E + (h0 + 1) * DH],
            W[E + h1 * DH:E + (h1 + 1) * DH],
        ], axis=0)
        # [P part=e_in, t, i] = rows[i, t*128 + e_in]
        wq_c = np.ascontiguousarray(
            rows_q.T.reshape(NT, P, P).transpose(1, 0, 2)
            .reshape(P, NT * P).astype(ml_dtypes.bfloat16))
        wk_c = rows_k.T.reshape(NT, P, P).transpose(1, 0, 2).reshape(P, NT * P)
        wv_f = np.zeros((E, 130), dtype=np.float32)
        wv_f[:, 0:DH] = W[2 * E + h0 * DH:2 * E + (h0 + 1) * DH].T
        wv_f[:, DH + 1:2 * DH + 1] = W[2 * E + h1 * DH:2 * E + (h1 + 1) * DH].T
        wv_c = wv_f.reshape(NT, P, 130).transpose(1, 0, 2).reshape(P, NT * 130)
        constsB_c = np.ascontiguousarray(np.concatenate(
            [wk_c, wv_c, cm2], axis=1).astype(ml_dtypes.bfloat16))

        brows = np.concatenate([
            b[h0 * DH:(h0 + 1) * DH], b[h1 * DH:(h1 + 1) * DH],
            b[E + h0 * DH:E + (h0 + 1) * DH],
            b[E + h1 * DH:E + (h1 + 1) * DH],
        ])
        bqk_c = brows.reshape(2, P).T  # [128, 2] f32
        bvr = np.zeros((260,), dtype=np.float32)
        for k in range(2):
            bvr[130 * k + 0:130 * k + DH] = b[
                2 * E + h0 * DH:2 * E + (h0 + 1) * DH]
            bvr[130 * k + DH] = 1.0
            bvr[130 * k + DH + 1:130 * k + 2 * DH + 1] = b[
                2 * E + h1 * DH:2 * E + (h1 + 1) * DH]
            bvr[130 * k + 2 * DH + 1] = 1.0
        bconsts_c = np.ascontiguousarray(np.concatenate(
            [bqk_c, np.broadcast_to(bvr, (P, 260))], axis=1).astype(
                np.float32))
        in_maps.append({
            "xt2": xt2, "constsA": wq_c, "constsB": constsB_c,
            "bconsts": bconsts_c,
        })
    return in_maps


def _run(x, l, W, b, trace=False):
    x = np.asarray(x, dtype=np.float32)
    lv = np.asarray(l).astype(np.int64)
    lpad, l2, order, offs, stot2, lex = _plan(lv)
    key = (lpad, order, lex)
    if key not in _cached:
        _cached[key] = _build_program(lpad, l2, order, offs, stot2, lex)
    nc = _cached[key]
    in_maps = _prepare_in_maps(x, lv, W, b, lpad, l2, order, offs, stot2)
    res = run_bass_kernel_spmd(nc, in_maps, list(range(8)), trace=trace)
    out = np.zeros((B, S, E), dtype=np.float32)
    # output tile offsets in `order`
    otile = {}
    ot = 0
    for bi in order:
        otile[bi] = ot
        ot += lpad[bi] // P
    ntiles = ot
    for c in range(8):
        oc = np.asarray(res.results[c]["o"], dtype=np.float32)
        # [P, ntiles, 130] -> rows in q order per tile
        ocq = oc.reshape(P, ntiles, 130).transpose(1, 0, 2).reshape(
            ntiles * P, 130)
        for bi in range(B):
            n = int(min(lv[bi], lpad[bi]))
            blk = ocq[otile[bi] * P:otile[bi] * P + n, :]
            for h in range(2):
                den = blk[:, h * 65 + DH:h * 65 + DH + 1]
                out[bi, :n, (2 * c + h) * DH:(2 * c + h + 1) * DH] = (
                    blk[:, h * 65:h * 65 + DH] / den)
    return out, res.exec_time_ns


def kernel(x, l, W, b):
    out, _ = _run(x, l, W, b, trace=False)
    return out


# revision 10
# speedup vs baseline: 1.4100x; 1.0649x over previous
"""Multi-head self-attention (B=8, S=1024, E=1024, H=16) on 8 TRN2 cores.

Sharding: tensor-parallel on heads — core c owns heads (2c, 2c+1) for ALL
batches. Every core runs the identical program (SPMD); only the W/bias column
slices differ per core, so the load is balanced by construction.

Per-batch sequence truncation: batch b is processed only up to
L_b = ceil(l_b/128)*128 rows (compile-time constants baked per call, cached on
the value of l). Rows q >= l_b are discarded on the host; causal masking makes
the padded key columns inside the last tile harmless for valid rows.

x lives in HBM in a blocked layout [128 part][blk][e_t][256] so that any
span of 256-blocks is one DMA with 128 large contiguous descriptors (the
per-row descriptor storm of the naive [E, stot] layout was both dispatch-
and queue-bound).

Engine plan per core:
  PE : QKV projection (contraction over E in 8 tiles of 128), scores QK^T
       (contraction 64 -> two PE row-halves in parallel), out = E@V with the
       softmax denominator as a ones column of V.
  ACT: exp only (one table set, loaded once at start), two heads per
       ACTIVATE instruction to amortize the fixed overhead.
  DVE: PSUM->SBUF copies (QK bias-add, paired V tiles with the V-bias fused
       into the copy, out rows), causal diag mask per k-tile.
  Out rows carry the denominator (65 cols per head); division on the host.
Emission software-pipelines proj(b+1) and the first score tile of (b+1) into
attention(b) to cover ACT latency and batch transitions.
"""

import sys

sys.path.insert(0, "/opt/trn_rl_repo")

from collections import deque

import numpy as np
import ml_dtypes

import concourse.bass as bass
import concourse.bacc as bacc
import concourse.mybir as mybir
import concourse.tile as tile
from concourse.bass import ds, ts
from concourse.bass_utils import run_bass_kernel_spmd

P = 128
BLK = 256
B, S, E, H = 8, 1024, 1024, 16
DH = E // H  # 64
F32 = mybir.dt.float32
BF16 = mybir.dt.bfloat16
ETDT = BF16
NT = E // P  # 8 contraction tiles

_cached = {}


def _plan(l):
    lex = [min(int(v), S) for v in l]
    lpad = [min((int(v) + P - 1) // P * P, S) for v in l]
    l2 = [(Lp + BLK - 1) // BLK * BLK for Lp in lpad]
    order = sorted(range(B), key=lambda b: -lpad[b])
    offs = {}   # in positions, within the padded (l2) layout
    off = 0
    for b in order:
        offs[b] = off
        off += l2[b]
    return tuple(lpad), tuple(l2), tuple(order), offs, off, tuple(lex)


def _build_program(lpad, l2, order, offs, stot2, lex):
    nc = bacc.Bacc(None, target_bir_lowering=False)

    # x: [128][blk][e_t][256] blocked layout (see module docstring)
    xt2 = nc.dram_tensor(
        "xt2", [P, (stot2 // BLK) * NT * BLK], BF16, kind="ExternalInput")[:]
    # constsA: wqk Q columns [P, NT, 128]
    constsA = nc.dram_tensor("constsA", [P, NT * P], BF16,
                             kind="ExternalInput")[:]
    # constsB: wqk K columns [P, NT*128] | wv [P, NT*130] | cm2 [P, 2*128]
    CB_W = NT * P + NT * 130 + 2 * P
    constsB = nc.dram_tensor("constsB", [P, CB_W], BF16,
                             kind="ExternalInput")[:]
    # bconsts: f32: cols 0:2 = qk bias (q, k); 2:262 = V bias row
    # replicated across partitions (with 1.0 denominator columns)
    bconsts = nc.dram_tensor("bconsts", [P, 262], F32, kind="ExternalInput")[:]
    # out: per batch tile t: [P, 130] (two heads x (64 + denom))
    o = nc.dram_tensor("o", [P, sum(Lp // P for Lp in lpad) * 130], F32,
                       kind="ExternalOutput")[:]
    # output tile offsets (in 130-col units) per batch, in `order`
    otile = {}
    ot = 0
    for b in order:
        otile[b] = ot
        ot += lpad[b] // P

    with tile.TileContext(nc) as tc:
        from contextlib import ExitStack

        with ExitStack() as ctx:
            sb = ctx.enter_context(tc.tile_pool(name="sb", bufs=1))
            wqkq_sb = sb.tile([P, NT, P], BF16)
            constsB_sb = sb.tile([P, CB_W], BF16)
            wqkk_sb = constsB_sb[:, 0:NT * P].rearrange(
                "p (t i) -> p t i", t=NT)
            wv_sb = constsB_sb[:, ds(NT * P, NT * 130)].rearrange(
                "p (t i) -> p t i", t=NT)
            cm2_sb = constsB_sb[:, ds(NT * P + NT * 130, 2 * P)].rearrange(
                "p (h q) -> p h q", h=2)
            bconsts_sb = sb.tile([P, 262], F32)
            bqk_sb = bconsts_sb[:, 0:2]
            bvr_sb = bconsts_sb[:, 2:262]
            warm_sb = sb.tile([P, 8], F32)
            warm2_sb = sb.tile([P, 8], BF16)

            xp = ctx.enter_context(tc.tile_pool(name="xp", bufs=1))
            qkp = ctx.enter_context(tc.tile_pool(name="qkp", bufs=2))
            vpp = ctx.enter_context(tc.tile_pool(name="vpp", bufs=2))
            eTp = ctx.enter_context(tc.tile_pool(name="eTp", bufs=2))
            outp = ctx.enter_context(tc.tile_pool(name="outp", bufs=4))
            pjps = ctx.enter_context(
                tc.tile_pool(name="pjps", bufs=2, space="PSUM"))
            sps_p = ctx.enter_context(
                tc.tile_pool(name="sps", bufs=2, space="PSUM"))
            ops_p = ctx.enter_context(
                tc.tile_pool(name="ops", bufs=2, space="PSUM"))

            # Warm the ACT exp table ASAP (overlaps const DMAs + first x DMA).
            nc.vector.memset(warm_sb, 0.0)
            nc.scalar.activation(
                out=warm2_sb, in_=warm_sb,
                func=mybir.ActivationFunctionType.Exp, scale=1.0)

            xbuf = {}

            def emit_xdma(b, eng, spans):
                """spans: list of (blk0, nblk) in 256-block units."""
                nb2 = l2[b] // BLK
                if b not in xbuf:
                    xbuf[b] = xp.tile([P, nb2, NT, BLK], BF16,
                                      name=f"xb{b}", tag=f"xb{b}")
                xb = xbuf[b]
                xsrc = xt2.rearrange("p (k t s) -> p k t s", t=NT, s=BLK)
                base = offs[b] // BLK
                for (a, n) in spans:
                    eng.dma_start(
                        out=xb[:, ds(a, n), :, :],
                        in_=xsrc[:, ds(base + a, n), :, :])

            # ---- DMA schedule ----
            # Consumers wait on the CUMULATIVE DMA stream of the dispatching
            # engine, and the 16 hw queues are FIFO across engines, so a
            # second dispatching engine would enqueue its (late-needed)
            # descriptors AHEAD of critical ones. Everything goes on sync,
            # strictly in consumer order.
            b0 = order[0]
            nb2_0 = l2[b0] // BLK
            nc.sync.dma_start(
                out=wqkq_sb, in_=constsA.rearrange("p (t i) -> p t i", t=NT))
            emit_xdma(b0, nc.sync, [(0, min(2, nb2_0))])
            nc.sync.dma_start(out=bconsts_sb, in_=bconsts)
            nc.sync.dma_start(out=constsB_sb, in_=constsB)
            if nb2_0 > 2:
                emit_xdma(b0, nc.sync, [(2, nb2_0 - 2)])
            for b in order[1:]:
                emit_xdma(b, nc.sync, [(0, l2[b] // BLK)])

            state = {}

            def qk_chunks(b):
                """(j, c0, cn) list; block-aligned starts, 512 preferred."""
                lq = lex[b]
                out = []
                for j, end in ((0, lq), (1, lpad[b])):
                    c0 = 0
                    while c0 < end:
                        cn = min(512, end - c0)
                        out.append((j, c0, cn))
                        c0 += cn
                return out

            def proj_thunks(b):
                """Closures emitting proj(b); each ~0.5-2us of PE work."""
                L = lpad[b]
                T = L // P
                qkT = qkp.tile([P, 2, L], BF16, name="qkT")
                vp = vpp.tile([P, T, 2, DH + 1], BF16, name="vp")
                toff = [t * L - t * (t - 1) * P // 2 for t in range(T + 1)]
                eT = eTp.tile([P, 2, toff[T]], ETDT, name="eT")
                out_sb = outp.tile([P, T, 130], F32, name="out_sb")
                state[b] = (qkT, vp, eT, out_sb, toff)

                def qk_chunk(j, c0, cn):
                    def th():
                        xb = xbuf[b]
                        ps = pjps.tile([P, 512], F32, name="pps")
                        # moving operand: whole blocks, then ragged tail
                        nwhole = (cn // BLK) * BLK
                        for e_t in range(NT):
                            if nwhole:
                                nc.tensor.matmul(
                                    ps[:, 0:nwhole],
                                    lhsT=(wqkq_sb if j == 0 else wqkk_sb)[
                                        :, e_t, :],
                                    rhs=xb[:, ds(c0 // BLK, nwhole // BLK),
                                           e_t, :],
                                    start=(e_t == 0),
                                    stop=(e_t == NT - 1 and cn == nwhole))
                        for e_t in range(NT):
                            if cn > nwhole:
                                nc.tensor.matmul(
                                    ps[:, ds(nwhole, cn - nwhole)],
                                    lhsT=(wqkq_sb if j == 0 else wqkk_sb)[
                                        :, e_t, :],
                                    rhs=xb[:, (c0 + nwhole) // BLK, e_t,
                                           ds(0, cn - nwhole)],
                                    start=(e_t == 0 and nwhole == 0),
                                    stop=(e_t == NT - 1),
                                    skip_group_check=True)
                        nc.vector.tensor_scalar_add(
                            qkT[:, j, ds(c0, cn)], ps[:, 0:cn],
                            bqk_sb[:, ds(j, 1)])
                    return th

                def v_pair(s_t, ns):
                    def th():
                        xb = xbuf[b]
                        ps = pjps.tile([P, 512], F32, name="pps")
                        for k in range(ns):
                            blk, half = (s_t + k) // 2, (s_t + k) % 2
                            for e_t in range(NT):
                                # start=True clears the whole PSUM bank, so
                                # only the very first matmul may set it.
                                nc.tensor.matmul(
                                    ps[:, ds(130 * k, 130)],
                                    lhsT=xb[:, blk, e_t, ds(half * P, P)],
                                    rhs=wv_sb[:, e_t, :],
                                    start=(k == 0 and e_t == 0),
                                    stop=(k == ns - 1 and e_t == NT - 1),
                                    skip_group_check=True)
                        nc.vector.tensor_tensor(
                            vp[:, ds(s_t, ns), :, :].rearrange(
                                "p s h d -> p (s h d)"),
                            ps[:, 0:130 * ns],
                            bvr_sb[:, 0:130 * ns],
                            mybir.AluOpType.add)
                    th.is_v = True
                    return th

                qk_list = [qk_chunk(*c) for c in qk_chunks(b)]
                v_list = [v_pair(s_t, min(2, T - s_t))
                          for s_t in range(0, T, 2)]
                thunks = []
                qi, vi = 0, 0
                while qi < len(qk_list) or vi < len(v_list):
                    if qi < len(qk_list):
                        thunks.append(qk_list[qi]); qi += 1
                    if vi < len(v_list):
                        thunks.append(v_list[vi]); vi += 1
                return thunks

            def emit_scores(b, t):
                """QK^T -> exp -> diag mask for k-tile t of batch b."""
                L = lpad[b]
                lq = lex[b]
                qkT, vp, eT, out_sb, toff = state[b]
                q0 = t * P
                qend = max(q0 + P, lq)
                chunks = [(c0, min(512, qend - c0))
                          for c0 in range(q0, qend, 512)]
                for ci, (c0, cn) in enumerate(chunks):
                    sps = sps_p.tile([P, 1024], F32, name="sps")
                    nc.tensor.matmul(
                        sps[:, 0:cn],
                        lhsT=qkT[0:DH, 1, ts(t, P)],
                        rhs=qkT[0:DH, 0, ds(c0, cn)],
                        start=True, stop=True)
                    nc.tensor.matmul(
                        sps[:, ds(512, cn)],
                        lhsT=qkT[DH:P, 1, ts(t, P)],
                        rhs=qkT[DH:P, 0, ds(c0, cn)],
                        start=True, stop=True)
                    nc.scalar.activation(
                        out=eT[:, :, ds(toff[t] + c0 - q0, cn)],
                        in_=sps.rearrange("p (h q) -> p h q", h=2)[
                            :, :, 0:cn],
                        func=mybir.ActivationFunctionType.Exp,
                        scale=1.0 / 32.0)
                    if ci == 0:
                        nc.vector.tensor_mul(
                            eT[:, :, ds(toff[t], P)],
                            eT[:, :, ds(toff[t], P)],
                            cm2_sb)

            def emit_attention(b, feeder, skip_t0, last):
                L = lpad[b]
                T = L // P
                qkT, vp, eT, out_sb, toff = state[b]
                nfeed0 = len(feeder)
                fed = 0

                def emit_out(t):
                    for h in range(2):
                        po = ops_p.tile([P, 512], F32, name="po")
                        for tk in range(t + 1):
                            nc.tensor.matmul(
                                po[:, 0:DH + 1],
                                lhsT=eT[:, h, ds(toff[tk] + (t - tk) * P, P)],
                                rhs=vp[:, tk, h, :],
                                start=(tk == 0),
                                stop=(tk == t))
                        nc.vector.tensor_copy(
                            out=out_sb[:, t, ds(h * 65, 65)],
                            in_=po[:, 0:DH + 1])

                for t in range(T):
                    if not (t == 0 and skip_t0):
                        emit_scores(b, t)
                    target = (t + 1) * nfeed0 // T
                    while fed < target and feeder:
                        feeder.popleft()(); fed += 1
                    emit_out(t)
                    if last:
                        nc.sync.dma_start(
                            out=o[:, ds((otile[b] + t) * 130, 130)],
                            in_=out_sb[:, t, :])
                if not last:
                    # one batch-sized DMA: per partition a single contiguous
                    # T*520B segment (per-row DMAs were descriptor-bound).
                    nc.sync.dma_start(
                        out=o[:, ds(otile[b] * 130, T * 130)],
                        in_=out_sb[:, 0:T, :].rearrange("p t d -> p (t d)"))
                while feeder:
                    feeder.popleft()()

            # ---- schedule ----
            for th in proj_thunks(order[0]):
                th()
            emit_scores(order[0], 0)
            for i, b in enumerate(order):
                feeder = deque()
                if i + 1 < B:
                    nb = order[i + 1]
                    feeder = deque(proj_thunks(nb))
                    feeder.append(lambda nb=nb: emit_scores(nb, 0))
                emit_attention(b, feeder, skip_t0=True, last=(i == B - 1))

    nc.compile()
    return nc


def _prepare_in_maps(x, l, W, b, lpad, l2, order, offs, stot2):
    W = np.asarray(W, dtype=np.float32)
    b = np.asarray(b, dtype=np.float32)

    # blocked x: xt2[p, blk, t, s] = x[batch, pos, e] with e = t*128+p,
    # pos = (blk - blk0_b)*256 + s
    nblk_tot = stot2 // BLK
    xt2 = np.empty((P, nblk_tot, NT, BLK), dtype=ml_dtypes.bfloat16)
    for bi in order:
        Lb2 = l2[bi]
        blk0 = offs[bi] // BLK
        # [L2, E] -> [nb2, 256, 8, 128] -> [128 p, nb2, 8 t, 256 s]
        xs = x[bi, :Lb2, :].reshape(Lb2 // BLK, BLK, NT, P).transpose(
            3, 0, 2, 1)
        xt2[:, blk0:blk0 + Lb2 // BLK] = xs.astype(ml_dtypes.bfloat16)
    xt2 = np.ascontiguousarray(xt2.reshape(P, nblk_tot * NT * BLK))

    k_idx = np.arange(P)[:, None]
    q_idx = np.arange(P)[None, :]
    cm = (k_idx <= q_idx).astype(np.float32)
    cm2 = np.concatenate([cm, cm], axis=1)  # [128, 256]

    in_maps = []
    for c in range(8):
        h0, h1 = 2 * c, 2 * c + 1
        rows_q = np.concatenate([
            W[h0 * DH:(h0 + 1) * DH],
            W[h1 * DH:(h1 + 1) * DH],
        ], axis=0)  # [128, E]
        rows_k = np.concatenate([
            W[E + h0 * DH:E + (h0 + 1) * DH],
            W[E + h1 * DH:E + (h1 + 1) * DH],
        ], axis=0)
        # [P part=e_in, t, i] = rows[i, t*128 + e_in]
        wq_c = np.ascontiguousarray(
            rows_q.T.reshape(NT, P, P).transpose(1, 0, 2)
            .reshape(P, NT * P).astype(ml_dtypes.bfloat16))
        wk_c = rows_k.T.reshape(NT, P, P).transpose(1, 0, 2).reshape(P, NT * P)
        wv_f = np.zeros((E, 130), dtype=np.float32)
        wv_f[:, 0:DH] = W[2 * E + h0 * DH:2 * E + (h0 + 1) * DH].T
        wv_f[:, DH + 1:2 * DH + 1] = W[2 * E + h1 * DH:2 * E + (h1 + 1) * DH].T
        wv_c = wv_f.reshape(NT, P, 130).transpose(1, 0, 2).reshape(P, NT * 130)
        constsB_c = np.ascontiguousarray(np.concatenate(
            [wk_c, wv_c, cm2], axis=1).astype(ml_dtypes.bfloat16))

        brows = np.concatenate([
            b[h0 * DH:(h0 + 1) * DH], b[h1 * DH:(h1 + 1) * DH],
            b[E + h0 * DH:E + (h0 + 1) * DH],
            b[E + h1 * DH:E + (h1 + 1) * DH],
        ])
        bqk_c = brows.reshape(2, P).T  # [128, 2] f32
        bvr = np.zeros((260,), dtype=np.float32)
        for k in range(2):
            bvr[130 * k + 0:130 * k + DH] = b[
                2 * E + h0 * DH:2 * E + (h0 + 1) * DH]
            bvr[130 * k + DH] = 1.0
            bvr[130 * k + DH + 1:130 * k + 2 * DH + 1] = b[
                2 * E + h1 * DH:2 * E + (h1 + 1) * DH]
            bvr[130 * k + 2 * DH + 1] = 1.0
        bconsts_c = np.ascontiguousarray(np.concatenate(
            [bqk_c, np.broadcast_to(bvr, (P, 260))], axis=1).astype(
                np.float32))
        in_maps.append({
            "xt2": xt2, "constsA": wq_c, "constsB": constsB_c,
            "bconsts": bconsts_c,
        })
    return in_maps


def _run(x, l, W, b, trace=False):
    x = np.asarray(x, dtype=np.float32)
    lv = np.asarray(l).astype(np.int64)
    lpad, l2, order, offs, stot2, lex = _plan(lv)
    key = (lpad, order, lex)
    if key not in _cached:
        _cached[key] = _build_program(lpad, l2, order, offs, stot2, lex)
    nc = _cached[key]
    in_maps = _prepare_in_maps(x, lv, W, b, lpad, l2, order, offs, stot2)
    res = run_bass_kernel_spmd(nc, in_maps, list(range(8)), trace=trace)
    out = np.zeros((B, S, E), dtype=np.float32)
    # output tile offsets in `order`
    otile = {}
    ot = 0
    for bi in order:
        otile[bi] = ot
        ot += lpad[bi] // P
    ntiles = ot
    for c in range(8):
        oc = np.asarray(res.results[c]["o"], dtype=np.float32)
        # [P, ntiles, 130] -> rows in q order per tile
        ocq = oc.reshape(P, ntiles, 130).transpose(1, 0, 2).reshape(
            ntiles * P, 130)
        for bi in range(B):
            n = int(min(lv[bi], lpad[bi]))
            blk = ocq[otile[bi] * P:otile[bi] * P + n, :]
            for h in range(2):
                den = blk[:, h * 65 + DH:h * 65 + DH + 1]
                out[bi, :n, (2 * c + h) * DH:(2 * c + h + 1) * DH] = (
                    blk[:, h * 65:h * 65 + DH] / den)
    return out, res.exec_time_ns


def kernel(x, l, W, b):
    out, _ = _run(x, l, W, b, trace=False)
    return out
